# revision 13
# baseline (speedup 1.0000x reference)
"""Multi-head GQA attention prefill (B=1, S=2048, D=4096, 32 q-heads /
8 kv-heads, head_dim=128, RoPE, causal) on 8 TRN2 NeuronCores.

Sharding: tensor-parallel over heads. Core c owns q-heads [4c, 4c+4) and
kv-head c (GQA group boundary == core boundary, so attention is fully
local). Out-projection sharded over wo rows (output columns): after
attention each core AllGathers the normalized attention outputs of all
heads and computes its 512 output columns; host concatenates.

Dataflow (transposed "P^T" form, bf16 matmuls, fp32 PSUM):
  qT/kT  [head_dim, S]  = proj(xT)            (RoPE'd on DVE)
  S^T    [Sk, Sq]       = kT_chunk.T @ qT     (causal blocks only)
  expS   bf16           = exp(S^T / sqrt(d))  (ScalarE, one [128,1024]
                          exp per TWO key chunks -- attention was
                          ScalarE-bound at one exp per chunk)
  oT     [head_dim, Sq] = sum_k V_chunk.T @ expS
  rowsum [1, Sq]        = ones.T @ expS       (PSUM accumulate)

v5 over v4 (trace: AllGather chain = trigger delay + 33-46us RDH + 9us
strided reload => out-proj fillers stalled the in-order PE stream):
  - AllGather split into head-pair halves, each fired mid-attention as
    soon as its two heads are normalized (norm runs inline per pair)
  - out-proj for superblock sb rides inside attn(sb+2), not attn(sb+1)
  - tail out-projs (sb=2,3) stream the gathered activations as
    contiguous [128,512] tiles (1KB/partition packets instead of 256B)
    and accumulate all four q-blocks at once in two 2-bank psums
  - RoPE in 4 DVE ops instead of 6: full-128-partition muls against
    partition-stacked cos/sin, then cross-base-partition add/sub
  - plus v2-v4: host-swizzled contiguous DMAs, batched x streaming,
    bf16 broadcast, diag trims, paired-exp, shared ones LDWEIGHTS
"""

import sys

sys.path.insert(0, "/opt/trn_rl_repo")

import numpy as np
import ml_dtypes

import concourse.bass as bass
import concourse.mybir as mybir
from concourse import bacc, tile
from concourse.bass_utils import run_bass_kernel_spmd
from concourse.masks import make_identity

F32 = mybir.dt.float32
BF16 = mybir.dt.bfloat16
BF16_NP = ml_dtypes.bfloat16

NCORES = 8
S = 2048
D = 4096
HD = 128                 # head dim
QH = 4                   # q heads per core
QROWS = QH * HD          # 512 q rows per core
SB = 512                 # seq superblock (free dim of most matmuls)
NSB = S // SB            # 4
DC = D // 128            # 32 contraction chunks
NKC = S // 128           # 16 key chunks
SCALE = 1.0 / np.sqrt(HD)


def build_graph():
    nc = bacc.Bacc("TRN2", target_bir_lowering=False, debug=False,
                   num_devices=NCORES)

    # host-swizzled inputs: partition-major, contiguous per partition
    x_sw = nc.declare_dram_parameter("x_sw", [128, DC, S], BF16, isOutput=False)
    wq_sw = nc.declare_dram_parameter("wq_sw", [128, QH, DC, HD], BF16,
                                      isOutput=False)
    wk_sw = nc.declare_dram_parameter("wk_sw", [128, DC, HD], BF16,
                                      isOutput=False)
    wv_sw = nc.declare_dram_parameter("wv_sw", [128, DC, HD], BF16,
                                      isOutput=False)
    wo_sw = nc.declare_dram_parameter("wo_sw", [128, DC, SB], BF16,
                                      isOutput=False)
    cosd = nc.declare_dram_parameter("cosd", [128, S], F32, isOutput=False)
    sind = nc.declare_dram_parameter("sind", [128, S], F32, isOutput=False)
    mask = nc.declare_dram_parameter("mask", [128, 128], BF16, isOutput=False)
    out = nc.declare_dram_parameter("out", [S, SB], F32, isOutput=True)

    # per-(superblock, head-pair) gather staging
    aT_loc = [[nc.dram_tensor(f"aT_loc{sb}_{g}", [2 * HD, SB], BF16)
               for g in range(2)] for sb in range(NSB)]
    aT_all = [[nc.dram_tensor(f"aT_all{sb}_{g}", [NCORES * 2 * HD, SB], BF16,
                              addr_space="Shared") for g in range(2)]
              for sb in range(NSB)]

    with tile.TileContext(nc) as tc:
        with tc.tile_pool(name="const", bufs=1) as cpool, \
             tc.tile_pool(name="wts", bufs=1) as wpool, \
             tc.tile_pool(name="kv", bufs=1) as kvpool, \
             tc.tile_pool(name="qt", bufs=8) as qpool, \
             tc.tile_pool(name="xs", bufs=6) as xpool, \
             tc.tile_pool(name="rope", bufs=2) as rpool, \
             tc.tile_pool(name="exps", bufs=3) as epool, \
             tc.tile_pool(name="onorm", bufs=2) as opool, \
             tc.tile_pool(name="ostream", bufs=3) as spool, \
             tc.tile_pool(name="af", bufs=4) as afpool, \
             tc.tile_pool(name="ps", bufs=1, space="PSUM") as ps:

            # ---- weights: first-needed chunks first ----
            wq_sb = wpool.tile([128, QH, DC, HD], BF16, tag="wq")
            wk_sb = wpool.tile([128, DC, HD], BF16, tag="wk")
            wv_sb = wpool.tile([128, DC, HD], BF16, tag="wv")
            wo_sb = wpool.tile([128, DC, SB], BF16, tag="wo")
            nc.scalar.dma_start(wk_sb[:, 0:4, :], wk_sw[:, 0:4, :])
            nc.scalar.dma_start(wv_sb[:, 0:4, :], wv_sw[:, 0:4, :])
            nc.scalar.dma_start(wq_sb[:, 0, 0:4, :], wq_sw[:, 0, 0:4, :])
            nc.scalar.dma_start(wk_sb[:, 4:DC, :], wk_sw[:, 4:DC, :])
            nc.scalar.dma_start(wv_sb[:, 4:DC, :], wv_sw[:, 4:DC, :])
            nc.scalar.dma_start(wq_sb[:, 0, 4:DC, :], wq_sw[:, 0, 4:DC, :])
            for h in range(1, QH):
                nc.scalar.dma_start(wq_sb[:, h, :, :], wq_sw[:, h, :, :])

            # ---- constants ----
            cos2 = cpool.tile([128, S], F32, tag="cos2")
            nc.gpsimd.dma_start(cos2[:], cosd[:, :])
            sin2 = cpool.tile([128, S], F32, tag="sin2")
            nc.gpsimd.dma_start(sin2[:], sind[:, :])
            mask_t = cpool.tile([128, 128], BF16, tag="mask")
            nc.gpsimd.dma_start(mask_t[:], mask[:])
            ident = cpool.tile([128, 128], BF16, tag="ident")
            make_identity(nc, ident[:])
            ones_col = cpool.tile([128, 1], BF16, tag="ones_col")
            nc.vector.memset(ones_col[:], 1.0)
            ones_row = cpool.tile([1, 128], BF16, tag="ones_row")
            nc.vector.memset(ones_row[:], 1.0)
            for g in range(0, DC, 8):
                nc.scalar.dma_start(wo_sb[:, g:g + 8, :], wo_sw[:, g:g + 8, :])

            # ---- persistent activations ----
            kT = kvpool.tile([128, S], BF16, tag="kT")
            v_sb = [kvpool.tile([128, HD], BF16, tag=f"v{kc}", name=f"v{kc}")
                    for kc in range(NKC)]

            qts = {}     # sb -> [qT tile per head]
            aus = {}     # (sb, h) -> unnormalized oT tile
            sums = {}    # sb -> rowsum collection tile
            acols = {}   # (sb, g, j) -> gathered-aT column block (strided)
            pts = {}     # sb -> preallocated transpose psums

            def ps2(name):
                return ps.tile([128, 2 * SB], F32, tag="pg2", bufs=2,
                               name=name)

            def ps1(name, shape=None, dtype=F32):
                return ps.tile(shape or [128, SB], dtype, tag="ps", bufs=3,
                               name=name)

            def emit_proj(sb):
                cols = bass.ts(sb, SB)
                pkv = ps2(f"pkv{sb}")
                pq0 = ps1(f"pq0_{sb}")
                pas_a = [(pkv[:, 0:SB], wk_sb, None),
                         (pkv[:, SB:2 * SB], wv_sb, None),
                         (pq0[:], wq_sb, 0)]
                pq12 = None
                pq3 = None
                for pi in range(2):
                    if pi == 1:
                        # preallocate transpose psums BEFORE pq3 so the
                        # rotation never makes a transpose wait on rope(q3)
                        pts[sb] = [ps1(f"pt{sb}_{j}", [128, 128], BF16)
                                   for j in range(4)]
                        pq12 = ps2(f"pq12_{sb}")
                        pq3 = ps1(f"pq3_{sb}")
                        pas = [(pq12[:, 0:SB], wq_sb, 1),
                               (pq12[:, SB:2 * SB], wq_sb, 2),
                               (pq3[:], wq_sb, 3)]
                    else:
                        pas = pas_a
                    for g in range(DC // 4):
                        xt = xpool.tile([128, 4, SB], BF16, tag="xt")
                        nc.sync.dma_start(xt[:], x_sw[:, 4 * g:4 * g + 4, cols])
                        for i in range(4):
                            dc = 4 * g + i
                            st, sp = dc == 0, dc == DC - 1
                            for (pt_, wt, h) in pas:
                                w = wt[:, dc, :] if h is None else \
                                    wt[:, h, dc, :]
                                nc.tensor.matmul(pt_, w, xt[:, i, :],
                                                 start=st, stop=sp)
                return pkv, pq0, pq12, pq3

            def rope(psum, dst, cols):
                # dst[e] = p[e]*cos - p[o]*sin ; dst[o] = p[o]*cos + p[e]*sin
                # cos2/sin2 carry cos/sin on BOTH partition halves: the cos
                # mul runs all 128 lanes in one op.  The sin muls write tb
                # half-SWAPPED; the cross-partition read is on the PSUM
                # operand, which the walrus verifier allows (SBUF operands
                # must share a start partition, PSUM need not).
                ta = rpool.tile([128, SB], F32, tag="rope_a")
                tb = rpool.tile([128, SB], F32, tag="rope_b")
                nc.vector.tensor_mul(ta[:], psum[:], cos2[:, cols])
                nc.vector.tensor_mul(tb[0:64, :], psum[64:128, :],
                                     sin2[0:64, cols])
                nc.vector.tensor_mul(tb[64:128, :], psum[0:64, :],
                                     sin2[64:128, cols])
                nc.vector.tensor_sub(dst[0:64, :], ta[0:64, :], tb[0:64, :])
                nc.vector.tensor_add(dst[64:128, :], ta[64:128, :],
                                     tb[64:128, :])

            def emit_rope_v(sb, tiles):
                pkv, pq0, pq12, pq3 = tiles
                cols = bass.ts(sb, SB)
                # vt copy first: it only needs pass A, runs during pass B
                vt = rpool.tile([128, SB], BF16, tag="vt")
                nc.vector.tensor_copy(vt[:], pkv[:, SB:2 * SB])
                rope(pkv[:, 0:SB], kT[:, cols], cols)
                qt_sb = [qpool.tile([128, SB], BF16, tag="qt",
                                    name=f"qT{sb}_{h}") for h in range(QH)]
                qts[sb] = qt_sb
                rope(pq0[:], qt_sb[0][:, :], cols)
                rope(pq12[:, 0:SB], qt_sb[1][:, :], cols)
                rope(pq12[:, SB:2 * SB], qt_sb[2][:, :], cols)
                rope(pq3[:], qt_sb[3][:, :], cols)
                for j in range(SB // 128):
                    pt = pts[sb][j]
                    nc.tensor.transpose(pt[:], vt[:, bass.ts(j, 128)],
                                        ident[:])
                    nc.vector.tensor_copy(v_sb[4 * sb + j][:], pt[:])

            def emit_norm_half(sb, g):
                """Normalize heads 2g, 2g+1 and fire their AllGather."""
                rec = opool.tile([64, SB], F32, tag="rec", bufs=2)
                nc.vector.reciprocal_approx_fast(rec[:], sums[sb][g][:])
                for e in range(2):
                    h = 2 * g + e
                    rc = opool.tile([1, SB], BF16, tag="rc", bufs=4)
                    nc.vector.tensor_copy(rc[:], rec[32 * e:32 * e + 1, :])
                    pb = ps1("pb")
                    nc.tensor.matmul(pb[:], ones_row[:], rc[:],
                                     start=True, stop=True)
                    at = opool.tile([128, SB], BF16, tag="at", bufs=4)
                    nc.vector.tensor_mul(at[:], aus[(sb, h)][:], pb[:])
                    nc.sync.dma_start(aT_loc[sb][g][bass.ts(e, 128), :],
                                      at[:])
                    del aus[(sb, h)]
                nc.gpsimd.collective_compute(
                    "AllGather",
                    mybir.AluOpType.bypass,
                    ins=[aT_loc[sb][g][:]],
                    outs=[aT_all[sb][g][:]],
                    replica_groups=[list(range(NCORES))],
                )

            def emit_acol_prefetch(sb):
                # strided prefetch for the in-attention out-proj fillers;
                # emitted inside attn(sb+1) so the 8MB of 256B-run traffic
                # does not starve the next projection's x stream
                for g in range(2):
                    for j in range(4):
                        acol = spool.tile([128, 16, 128], BF16, tag="acol",
                                          bufs=4, name=f"acol{sb}_{g}_{j}")
                        nc.gpsimd.dma_start(
                            acol[:],
                            aT_all[sb][g][:, bass.ts(j, 128)].rearrange(
                                "(c p) m -> p c m", p=128))
                        acols[(sb, g, j)] = acol

            def emit_attn(sb, filler=None, prefetch=None):
                def fill():
                    if filler is not None:
                        next(filler, None)
                nkc = 4 * sb + 4
                npair = nkc // 2
                sm = [opool.tile([64, SB], F32, tag="sums", bufs=4,
                                 name=f"sums{sb}_{g}") for g in range(2)]
                nc.gpsimd.memset(sm[0][:], 1.0)
                nc.gpsimd.memset(sm[1][:], 1.0)
                sums[sb] = sm

                def c0_of(kc):
                    j = kc - 4 * sb
                    return 128 * j if j > 0 else 0

                for h in range(QH):
                    po = ps1("po")
                    psum = ps1("psum", [1, SB])
                    es = {}

                    def qk2(p):
                        # two key chunks -> one 2-bank score tile -> ONE exp
                        pg2 = ps2("pg2")
                        e2 = epool.tile([128, 2 * SB], BF16, tag="es")
                        for half in (0, 1):
                            kc = 2 * p + half
                            c0 = c0_of(kc)
                            b = half * SB
                            nc.tensor.matmul(
                                pg2[:, b + c0:b + SB], kT[:, bass.ts(kc, 128)],
                                qts[sb][h][:, c0:SB], start=True, stop=True)
                        cl = c0_of(2 * p)
                        nc.scalar.activation(e2[:, cl:2 * SB],
                                             pg2[:, cl:2 * SB],
                                             mybir.ActivationFunctionType.Exp,
                                             scale=SCALE)
                        for half in (0, 1):
                            kc = 2 * p + half
                            j = kc - 4 * sb
                            if j >= 0:
                                lo = half * SB + 128 * j
                                nc.vector.tensor_mul(e2[:, lo:lo + 128],
                                                     e2[:, lo:lo + 128],
                                                     mask_t[:])
                        es[p] = e2

                    def pv_ones(p):
                        # both PVs then both rowsums: the two rowsum matmuls
                        # share one ones_col LDWEIGHTS
                        e2 = es[p]
                        for half in (0, 1):
                            kc = 2 * p + half
                            c0 = c0_of(kc)
                            st, sp = kc == 0, kc == nkc - 1
                            nc.tensor.matmul(po[:, c0:SB], v_sb[kc][:],
                                             e2[:, half * SB + c0:
                                                 half * SB + SB],
                                             start=st, stop=sp)
                        for half in (0, 1):
                            kc = 2 * p + half
                            c0 = c0_of(kc)
                            st, sp = kc == 0, kc == nkc - 1
                            nc.tensor.matmul(psum[:, c0:SB], ones_col[:],
                                             e2[:, half * SB + c0:
                                                 half * SB + SB],
                                             start=st, stop=sp)
                        del es[p]
                        fill()

                    qk2(0)
                    for p in range(1, npair):
                        qk2(p)
                        pv_ones(p - 1)
                    pv_ones(npair - 1)

                    nc.vector.tensor_copy(
                        sm[h // 2][32 * (h % 2):32 * (h % 2) + 1, :], psum[:])
                    au = opool.tile([128, SB], BF16, tag="au", bufs=8,
                                    name=f"au{sb}_{h}")
                    nc.vector.tensor_copy(au[:], po[:])
                    aus[(sb, h)] = au
                    if h == 0 and prefetch is not None:
                        prefetch()
                    if h == 1:
                        emit_norm_half(sb, 0)
                    elif h == 3:
                        emit_norm_half(sb, 1)

            def outproj_steps(sbp, group=3):
                """Filler generator for in-attention out-proj (sb 0/1)."""
                for j in range(4):
                    mc = 4 * sbp + j
                    pout = ps.tile([128, SB], F32, tag="pout", bufs=1,
                                   name="pout")
                    n = 0
                    for g in range(2):
                        acol = acols.pop((sbp, g, j))
                        for idx in range(16):
                            nc.tensor.matmul(
                                pout[:], acol[:, idx, :],
                                wo_sb[:, 16 * g + idx, :],
                                start=(g == 0 and idx == 0),
                                stop=(g == 1 and idx == 15))
                            n += 1
                            if n % group == 0:
                                yield
                    ot = spool.tile([128, SB], F32, tag="ot")
                    nc.vector.tensor_copy(ot[:], pout[:])
                    nc.sync.dma_start(out[bass.ts(mc, 128), :], ot[:])

            def emit_outproj_tail(sbp, queue):
                """Tail out-proj: contiguous acf streaming, all four
                q-blocks accumulate at once in two 2-bank psums."""
                pouts = [ps2(f"pout2_{sbp}_{jj}") for jj in range(2)]
                for g in range(2):
                    for idx in range(16):
                        acf = afpool.tile([128, SB], BF16, tag="acf")
                        queue.dma_start(acf[:],
                                        aT_all[sbp][g][bass.ts(idx, 128), :])
                        st = g == 0 and idx == 0
                        sp = g == 1 and idx == 15
                        for j in range(4):
                            pout = pouts[j // 2][:, (j % 2) * SB:
                                                 (j % 2) * SB + SB]
                            nc.tensor.matmul(pout, acf[:, bass.ts(j, 128)],
                                             wo_sb[:, 16 * g + idx, :],
                                             start=st, stop=sp)
                for j in range(4):
                    mc = 4 * sbp + j
                    ot = spool.tile([128, SB], F32, tag="ot")
                    nc.vector.tensor_copy(
                        ot[:], pouts[j // 2][:, (j % 2) * SB:(j % 2) * SB + SB])
                    nc.sync.dma_start(out[bass.ts(mc, 128), :], ot[:])

            def drain(gen):
                for _ in gen:
                    pass

            # ---- schedule ----
            t0_ = emit_proj(0)
            emit_rope_v(0, t0_)
            emit_attn(0)
            t1_ = emit_proj(1)
            emit_rope_v(1, t1_)
            emit_attn(1, prefetch=lambda: emit_acol_prefetch(0))
            t2_ = emit_proj(2)
            emit_rope_v(2, t2_)
            go0 = outproj_steps(0)
            emit_attn(2, go0, prefetch=lambda: emit_acol_prefetch(1))
            drain(go0)
            t3_ = emit_proj(3)
            emit_rope_v(3, t3_)
            go1 = outproj_steps(1)
            emit_attn(3, go1)
            drain(go1)
            emit_outproj_tail(2, nc.sync)
            emit_outproj_tail(3, nc.scalar)

    nc.compile()
    return nc


_PERM = np.concatenate([np.arange(0, HD, 2), np.arange(1, HD, 2)])


def _sw3(a, free):
    """[D, free] row-major -> [128, DC, free] partition-major contiguous."""
    return np.ascontiguousarray(
        a.reshape(DC, 128, free).transpose(1, 0, 2)).astype(BF16_NP)


def _prep_inputs(x, wq, wk, wv, wo, freqs_cos, freqs_sin):
    xT = np.ascontiguousarray(x.reshape(S, D).T)
    x_sw = _sw3(xT, S)
    cosT = np.ascontiguousarray(freqs_cos.T).astype(np.float32)
    sinT = np.ascontiguousarray(freqs_sin.T).astype(np.float32)
    cosd = np.ascontiguousarray(np.vstack([cosT, cosT]))
    sind = np.ascontiguousarray(np.vstack([sinT, sinT]))
    mask = np.triu(np.ones((128, 128), dtype=np.float32)).astype(BF16_NP)

    # gathered chunk order for the split AllGather: (g, c, e) with h = 2g+e
    order = [c * 4 + 2 * g + e
             for g in range(2) for c in range(NCORES) for e in range(2)]

    qperm = np.concatenate([h * HD + _PERM for h in range(QH)])
    in_maps = []
    for c in range(NCORES):
        wq_c = wq[c * QROWS:(c + 1) * QROWS][qperm]     # [512, D]
        wk_c = wk[c * HD:(c + 1) * HD][_PERM]           # [128, D]
        wv_c = wv[c * HD:(c + 1) * HD]                  # [128, D]
        wo_c = wo[c * SB:(c + 1) * SB]                  # [512, D]
        wqT_c = wq_c.T                                  # [D, 512]
        wq_sw = np.ascontiguousarray(
            wqT_c.reshape(DC, 128, QH, HD).transpose(1, 2, 0, 3)
        ).astype(BF16_NP)
        woT_chunks = wo_c.T.reshape(DC, 128, SB)        # [oldchunk, p, m]
        wo_sw = np.ascontiguousarray(
            woT_chunks[order].transpose(1, 0, 2)).astype(BF16_NP)
        in_maps.append({
            "x_sw": x_sw,
            "wq_sw": wq_sw,
            "wk_sw": _sw3(wk_c.T, HD),
            "wv_sw": _sw3(wv_c.T, HD),
            "wo_sw": wo_sw,
            "cosd": cosd,
            "sind": sind,
            "mask": mask,
        })
    return in_maps


def kernel(x, wq, wk, wv, wo, freqs_cos, freqs_sin, start_pos=0, *,
           _trace=False):
    in_maps = _prep_inputs(np.asarray(x, np.float32), np.asarray(wq, np.float32),
                           np.asarray(wk, np.float32), np.asarray(wv, np.float32),
                           np.asarray(wo, np.float32),
                           np.asarray(freqs_cos, np.float32),
                           np.asarray(freqs_sin, np.float32))
    nc = build_graph()
    res = run_bass_kernel_spmd(nc, in_maps, core_ids=list(range(NCORES)),
                               trace=_trace)
    full = np.concatenate([res.results[c]["out"] for c in range(NCORES)],
                          axis=1)
    out = full.reshape(1, S, D).astype(np.float32)
    if _trace:
        return out, res
    return out


# revision 14
# speedup vs baseline: 1.0014x; 1.0014x over previous
"""Multi-head GQA attention prefill (B=1, S=2048, D=4096, 32 q-heads /
8 kv-heads, head_dim=128, RoPE, causal) on 8 TRN2 NeuronCores.

Sharding: tensor-parallel over heads. Core c owns q-heads [4c, 4c+4) and
kv-head c (GQA group boundary == core boundary, so attention is fully
local). Out-projection sharded over wo rows (output columns): after
attention each core AllGathers the normalized attention outputs of all
heads and computes its 512 output columns; host concatenates.

Dataflow (transposed "P^T" form, bf16 matmuls, fp32 PSUM):
  qT/kT  [head_dim, S]  = proj(xT)            (RoPE'd on DVE)
  S^T    [Sk, Sq]       = kT_chunk.T @ qT     (causal blocks only)
  expS   bf16           = exp(S^T / sqrt(d))  (ScalarE, one [128,1024]
                          exp per TWO key chunks -- attention was
                          ScalarE-bound at one exp per chunk)
  oT     [head_dim, Sq] = sum_k V_chunk.T @ expS
  rowsum [1, Sq]        = ones.T @ expS       (PSUM accumulate)

v5 over v4 (trace: AllGather chain = trigger delay + 33-46us RDH + 9us
strided reload => out-proj fillers stalled the in-order PE stream):
  - AllGather split into head-pair halves, each fired mid-attention as
    soon as its two heads are normalized (norm runs inline per pair)
  - out-proj for superblock sb rides inside attn(sb+2), not attn(sb+1)
  - tail out-projs (sb=2,3) stream the gathered activations as
    contiguous [128,512] tiles (1KB/partition packets instead of 256B)
    and accumulate all four q-blocks at once in two 2-bank psums
  - RoPE in 4 DVE ops instead of 6: full-128-partition muls against
    partition-stacked cos/sin, then cross-base-partition add/sub
  - plus v2-v4: host-swizzled contiguous DMAs, batched x streaming,
    bf16 broadcast, diag trims, paired-exp, shared ones LDWEIGHTS
"""

import sys

sys.path.insert(0, "/opt/trn_rl_repo")

import numpy as np
import ml_dtypes

import concourse.bass as bass
import concourse.mybir as mybir
from concourse import bacc, tile
from concourse.bass_utils import run_bass_kernel_spmd
from concourse.masks import make_identity

F32 = mybir.dt.float32
BF16 = mybir.dt.bfloat16
BF16_NP = ml_dtypes.bfloat16

NCORES = 8
S = 2048
D = 4096
HD = 128                 # head dim
QH = 4                   # q heads per core
QROWS = QH * HD          # 512 q rows per core
SB = 512                 # seq superblock (free dim of most matmuls)
NSB = S // SB            # 4
DC = D // 128            # 32 contraction chunks
NKC = S // 128           # 16 key chunks
SCALE = 1.0 / np.sqrt(HD)


def build_graph():
    nc = bacc.Bacc("TRN2", target_bir_lowering=False, debug=False,
                   num_devices=NCORES)

    # host-swizzled inputs: partition-major, contiguous per partition
    x_sw = nc.declare_dram_parameter("x_sw", [128, DC, S], BF16, isOutput=False)
    wq_sw = nc.declare_dram_parameter("wq_sw", [128, QH, DC, HD], BF16,
                                      isOutput=False)
    wk_sw = nc.declare_dram_parameter("wk_sw", [128, DC, HD], BF16,
                                      isOutput=False)
    wv_sw = nc.declare_dram_parameter("wv_sw", [128, DC, HD], BF16,
                                      isOutput=False)
    wo_sw = nc.declare_dram_parameter("wo_sw", [128, DC, SB], BF16,
                                      isOutput=False)
    cosd = nc.declare_dram_parameter("cosd", [128, S], F32, isOutput=False)
    sind = nc.declare_dram_parameter("sind", [128, S], F32, isOutput=False)
    mask = nc.declare_dram_parameter("mask", [128, 128], BF16, isOutput=False)
    out = nc.declare_dram_parameter("out", [S, SB], F32, isOutput=True)

    # per-(superblock, head-pair) gather staging
    aT_loc = [[nc.dram_tensor(f"aT_loc{sb}_{g}", [2 * HD, SB], BF16)
               for g in range(2)] for sb in range(NSB)]
    aT_all = [[nc.dram_tensor(f"aT_all{sb}_{g}", [NCORES * 2 * HD, SB], BF16,
                              addr_space="Shared") for g in range(2)]
              for sb in range(NSB)]

    with tile.TileContext(nc) as tc:
        with tc.tile_pool(name="const", bufs=1) as cpool, \
             tc.tile_pool(name="wts", bufs=1) as wpool, \
             tc.tile_pool(name="kv", bufs=1) as kvpool, \
             tc.tile_pool(name="qt", bufs=8) as qpool, \
             tc.tile_pool(name="xs", bufs=6) as xpool, \
             tc.tile_pool(name="rope", bufs=2) as rpool, \
             tc.tile_pool(name="exps", bufs=4) as epool, \
             tc.tile_pool(name="onorm", bufs=2) as opool, \
             tc.tile_pool(name="ostream", bufs=3) as spool, \
             tc.tile_pool(name="af", bufs=8) as afpool, \
             tc.tile_pool(name="ps", bufs=1, space="PSUM") as ps:

            # ---- weights: first-needed chunks first ----
            wq_sb = wpool.tile([128, QH, DC, HD], BF16, tag="wq")
            wk_sb = wpool.tile([128, DC, HD], BF16, tag="wk")
            wv_sb = wpool.tile([128, DC, HD], BF16, tag="wv")
            wo_sb = wpool.tile([128, DC, SB], BF16, tag="wo")
            nc.scalar.dma_start(wk_sb[:, 0:4, :], wk_sw[:, 0:4, :])
            nc.scalar.dma_start(wv_sb[:, 0:4, :], wv_sw[:, 0:4, :])
            nc.scalar.dma_start(wq_sb[:, 0, 0:4, :], wq_sw[:, 0, 0:4, :])
            nc.scalar.dma_start(wk_sb[:, 4:DC, :], wk_sw[:, 4:DC, :])
            nc.scalar.dma_start(wv_sb[:, 4:DC, :], wv_sw[:, 4:DC, :])
            nc.scalar.dma_start(wq_sb[:, 0, 4:DC, :], wq_sw[:, 0, 4:DC, :])
            for h in range(1, QH):
                nc.scalar.dma_start(wq_sb[:, h, :, :], wq_sw[:, h, :, :])

            # ---- constants ----
            cos2 = cpool.tile([128, S], F32, tag="cos2")
            nc.gpsimd.dma_start(cos2[:], cosd[:, :])
            sin2 = cpool.tile([128, S], F32, tag="sin2")
            nc.gpsimd.dma_start(sin2[:], sind[:, :])
            mask_t = cpool.tile([128, 128], BF16, tag="mask")
            nc.gpsimd.dma_start(mask_t[:], mask[:])
            ident = cpool.tile([128, 128], BF16, tag="ident")
            make_identity(nc, ident[:])
            ones_col = cpool.tile([128, 1], BF16, tag="ones_col")
            nc.vector.memset(ones_col[:], 1.0)
            ones_row = cpool.tile([1, 128], BF16, tag="ones_row")
            nc.vector.memset(ones_row[:], 1.0)
            for g in range(0, DC, 8):
                nc.scalar.dma_start(wo_sb[:, g:g + 8, :], wo_sw[:, g:g + 8, :])

            # ---- persistent activations ----
            kT = kvpool.tile([128, S], BF16, tag="kT")
            v_sb = [kvpool.tile([128, HD], BF16, tag=f"v{kc}", name=f"v{kc}")
                    for kc in range(NKC)]

            qts = {}     # sb -> [qT tile per head]
            aus = {}     # (sb, h) -> unnormalized oT tile
            sums = {}    # sb -> rowsum collection tile
            acols = {}   # (sb, g, j) -> gathered-aT column block (strided)
            pts = {}     # sb -> preallocated transpose psums

            def ps2(name):
                return ps.tile([128, 2 * SB], F32, tag="pg2", bufs=2,
                               name=name)

            def ps1(name, shape=None, dtype=F32):
                return ps.tile(shape or [128, SB], dtype, tag="ps", bufs=3,
                               name=name)

            def emit_proj(sb):
                cols = bass.ts(sb, SB)
                pkv = ps2(f"pkv{sb}")
                pq0 = ps1(f"pq0_{sb}")
                pas_a = [(pkv[:, 0:SB], wk_sb, None),
                         (pkv[:, SB:2 * SB], wv_sb, None),
                         (pq0[:], wq_sb, 0)]
                pq12 = None
                pq3 = None
                for pi in range(2):
                    if pi == 1:
                        # preallocate transpose psums BEFORE pq3 so the
                        # rotation never makes a transpose wait on rope(q3)
                        pts[sb] = [ps1(f"pt{sb}_{j}", [128, 128], BF16)
                                   for j in range(4)]
                        pq12 = ps2(f"pq12_{sb}")
                        pq3 = ps1(f"pq3_{sb}")
                        pas = [(pq12[:, 0:SB], wq_sb, 1),
                               (pq12[:, SB:2 * SB], wq_sb, 2),
                               (pq3[:], wq_sb, 3)]
                    else:
                        pas = pas_a
                    for g in range(DC // 4):
                        xt = xpool.tile([128, 4, SB], BF16, tag="xt")
                        nc.sync.dma_start(xt[:], x_sw[:, 4 * g:4 * g + 4, cols])
                        for i in range(4):
                            dc = 4 * g + i
                            st, sp = dc == 0, dc == DC - 1
                            for (pt_, wt, h) in pas:
                                w = wt[:, dc, :] if h is None else \
                                    wt[:, h, dc, :]
                                nc.tensor.matmul(pt_, w, xt[:, i, :],
                                                 start=st, stop=sp)
                return pkv, pq0, pq12, pq3

            def rope(psum, dst, cols):
                # dst[e] = p[e]*cos - p[o]*sin ; dst[o] = p[o]*cos + p[e]*sin
                # cos2/sin2 carry cos/sin on BOTH partition halves: the cos
                # mul runs all 128 lanes in one op.  The sin muls write tb
                # half-SWAPPED; the cross-partition read is on the PSUM
                # operand, which the walrus verifier allows (SBUF operands
                # must share a start partition, PSUM need not).
                ta = rpool.tile([128, SB], F32, tag="rope_a")
                tb = rpool.tile([128, SB], F32, tag="rope_b")
                nc.vector.tensor_mul(ta[:], psum[:], cos2[:, cols])
                nc.vector.tensor_mul(tb[0:64, :], psum[64:128, :],
                                     sin2[0:64, cols])
                nc.vector.tensor_mul(tb[64:128, :], psum[0:64, :],
                                     sin2[64:128, cols])
                nc.vector.tensor_sub(dst[0:64, :], ta[0:64, :], tb[0:64, :])
                nc.vector.tensor_add(dst[64:128, :], ta[64:128, :],
                                     tb[64:128, :])

            def emit_rope_v(sb, tiles):
                pkv, pq0, pq12, pq3 = tiles
                cols = bass.ts(sb, SB)
                # vt copy first: it only needs pass A, runs during pass B
                vt = rpool.tile([128, SB], BF16, tag="vt")
                nc.vector.tensor_copy(vt[:], pkv[:, SB:2 * SB])
                rope(pkv[:, 0:SB], kT[:, cols], cols)
                qt_sb = [qpool.tile([128, SB], BF16, tag="qt",
                                    name=f"qT{sb}_{h}") for h in range(QH)]
                qts[sb] = qt_sb
                rope(pq0[:], qt_sb[0][:, :], cols)
                rope(pq12[:, 0:SB], qt_sb[1][:, :], cols)
                rope(pq12[:, SB:2 * SB], qt_sb[2][:, :], cols)
                rope(pq3[:], qt_sb[3][:, :], cols)
                for j in range(SB // 128):
                    pt = pts[sb][j]
                    nc.tensor.transpose(pt[:], vt[:, bass.ts(j, 128)],
                                        ident[:])
                    nc.vector.tensor_copy(v_sb[4 * sb + j][:], pt[:])

            def emit_norm_half(sb, g):
                """Normalize heads 2g, 2g+1 and fire their AllGather."""
                rec = opool.tile([64, SB], F32, tag="rec", bufs=2)
                nc.vector.reciprocal_approx_fast(rec[:], sums[sb][g][:])
                for e in range(2):
                    h = 2 * g + e
                    rc = opool.tile([1, SB], BF16, tag="rc", bufs=4)
                    nc.vector.tensor_copy(rc[:], rec[32 * e:32 * e + 1, :])
                    pb = ps1("pb")
                    nc.tensor.matmul(pb[:], ones_row[:], rc[:],
                                     start=True, stop=True)
                    at = opool.tile([128, SB], BF16, tag="at", bufs=4)
                    nc.vector.tensor_mul(at[:], aus[(sb, h)][:], pb[:])
                    nc.sync.dma_start(aT_loc[sb][g][bass.ts(e, 128), :],
                                      at[:])
                    del aus[(sb, h)]
                nc.gpsimd.collective_compute(
                    "AllGather",
                    mybir.AluOpType.bypass,
                    ins=[aT_loc[sb][g][:]],
                    outs=[aT_all[sb][g][:]],
                    replica_groups=[list(range(NCORES))],
                )

            def emit_attn(sb):
                nkc = 4 * sb + 4
                npair = nkc // 2
                sm = [opool.tile([64, SB], F32, tag="sums", bufs=4,
                                 name=f"sums{sb}_{g}") for g in range(2)]
                nc.gpsimd.memset(sm[0][:], 1.0)
                nc.gpsimd.memset(sm[1][:], 1.0)
                sums[sb] = sm

                def c0_of(kc):
                    j = kc - 4 * sb
                    return 128 * j if j > 0 else 0

                for h in range(QH):
                    po = ps1("po")
                    psum = ps1("psum", [1, SB])
                    es = {}

                    def qk2(p):
                        # two key chunks -> one 2-bank score tile -> ONE exp
                        pg2 = ps2("pg2")
                        e2 = epool.tile([128, 2 * SB], BF16, tag="es")
                        for half in (0, 1):
                            kc = 2 * p + half
                            c0 = c0_of(kc)
                            b = half * SB
                            nc.tensor.matmul(
                                pg2[:, b + c0:b + SB], kT[:, bass.ts(kc, 128)],
                                qts[sb][h][:, c0:SB], start=True, stop=True)
                        cl = c0_of(2 * p)
                        nc.scalar.activation(e2[:, cl:2 * SB],
                                             pg2[:, cl:2 * SB],
                                             mybir.ActivationFunctionType.Exp,
                                             scale=SCALE)
                        for half in (0, 1):
                            kc = 2 * p + half
                            j = kc - 4 * sb
                            if j >= 0:
                                lo = half * SB + 128 * j
                                nc.vector.tensor_mul(e2[:, lo:lo + 128],
                                                     e2[:, lo:lo + 128],
                                                     mask_t[:])
                        es[p] = e2

                    def pv_ones(p):
                        # both PVs then both rowsums: the two rowsum matmuls
                        # share one ones_col LDWEIGHTS
                        e2 = es[p]
                        for half in (0, 1):
                            kc = 2 * p + half
                            c0 = c0_of(kc)
                            st, sp = kc == 0, kc == nkc - 1
                            nc.tensor.matmul(po[:, c0:SB], v_sb[kc][:],
                                             e2[:, half * SB + c0:
                                                 half * SB + SB],
                                             start=st, stop=sp)
                        for half in (0, 1):
                            kc = 2 * p + half
                            c0 = c0_of(kc)
                            st, sp = kc == 0, kc == nkc - 1
                            nc.tensor.matmul(psum[:, c0:SB], ones_col[:],
                                             e2[:, half * SB + c0:
                                                 half * SB + SB],
                                             start=st, stop=sp)
                        del es[p]

                    qk2(0)
                    for p in range(1, npair):
                        qk2(p)
                        pv_ones(p - 1)
                    pv_ones(npair - 1)

                    nc.vector.tensor_copy(
                        sm[h // 2][32 * (h % 2):32 * (h % 2) + 1, :], psum[:])
                    au = opool.tile([128, SB], BF16, tag="au", bufs=8,
                                    name=f"au{sb}_{h}")
                    nc.vector.tensor_copy(au[:], po[:])
                    aus[(sb, h)] = au
                    if h == 1:
                        emit_norm_half(sb, 0)
                    elif h == 3:
                        emit_norm_half(sb, 1)

            def emit_outproj_tail(sbp, queue):
                """Tail out-proj: contiguous acf streaming, all four
                q-blocks accumulate at once in two 2-bank psums."""
                pouts = [ps2(f"pout2_{sbp}_{jj}") for jj in range(2)]
                for g in range(2):
                    for idx in range(16):
                        acf = afpool.tile([128, SB], BF16, tag="acf")
                        queue.dma_start(acf[:],
                                        aT_all[sbp][g][bass.ts(idx, 128), :])
                        st = g == 0 and idx == 0
                        sp = g == 1 and idx == 15
                        for j in range(4):
                            pout = pouts[j // 2][:, (j % 2) * SB:
                                                 (j % 2) * SB + SB]
                            nc.tensor.matmul(pout, acf[:, bass.ts(j, 128)],
                                             wo_sb[:, 16 * g + idx, :],
                                             start=st, stop=sp)
                for j in range(4):
                    mc = 4 * sbp + j
                    ot = spool.tile([128, SB], F32, tag="ot")
                    nc.vector.tensor_copy(
                        ot[:], pouts[j // 2][:, (j % 2) * SB:(j % 2) * SB + SB])
                    nc.sync.dma_start(out[bass.ts(mc, 128), :], ot[:])

            # ---- schedule ----
            t0_ = emit_proj(0)
            emit_rope_v(0, t0_)
            emit_attn(0)
            t1_ = emit_proj(1)
            emit_rope_v(1, t1_)
            emit_attn(1)
            t2_ = emit_proj(2)
            emit_rope_v(2, t2_)
            emit_attn(2)
            t3_ = emit_proj(3)
            emit_rope_v(3, t3_)
            emit_attn(3)
            emit_outproj_tail(0, nc.sync)
            emit_outproj_tail(1, nc.scalar)
            emit_outproj_tail(2, nc.sync)
            emit_outproj_tail(3, nc.scalar)

    nc.compile()
    return nc


_PERM = np.concatenate([np.arange(0, HD, 2), np.arange(1, HD, 2)])


def _sw3(a, free):
    """[D, free] row-major -> [128, DC, free] partition-major contiguous."""
    return np.ascontiguousarray(
        a.reshape(DC, 128, free).transpose(1, 0, 2)).astype(BF16_NP)


def _prep_inputs(x, wq, wk, wv, wo, freqs_cos, freqs_sin):
    xT = np.ascontiguousarray(x.reshape(S, D).T)
    x_sw = _sw3(xT, S)
    cosT = np.ascontiguousarray(freqs_cos.T).astype(np.float32)
    sinT = np.ascontiguousarray(freqs_sin.T).astype(np.float32)
    cosd = np.ascontiguousarray(np.vstack([cosT, cosT]))
    sind = np.ascontiguousarray(np.vstack([sinT, sinT]))
    mask = np.triu(np.ones((128, 128), dtype=np.float32)).astype(BF16_NP)

    # gathered chunk order for the split AllGather: (g, c, e) with h = 2g+e
    order = [c * 4 + 2 * g + e
             for g in range(2) for c in range(NCORES) for e in range(2)]

    qperm = np.concatenate([h * HD + _PERM for h in range(QH)])
    in_maps = []
    for c in range(NCORES):
        wq_c = wq[c * QROWS:(c + 1) * QROWS][qperm]     # [512, D]
        wk_c = wk[c * HD:(c + 1) * HD][_PERM]           # [128, D]
        wv_c = wv[c * HD:(c + 1) * HD]                  # [128, D]
        wo_c = wo[c * SB:(c + 1) * SB]                  # [512, D]
        wqT_c = wq_c.T                                  # [D, 512]
        wq_sw = np.ascontiguousarray(
            wqT_c.reshape(DC, 128, QH, HD).transpose(1, 2, 0, 3)
        ).astype(BF16_NP)
        woT_chunks = wo_c.T.reshape(DC, 128, SB)        # [oldchunk, p, m]
        wo_sw = np.ascontiguousarray(
            woT_chunks[order].transpose(1, 0, 2)).astype(BF16_NP)
        in_maps.append({
            "x_sw": x_sw,
            "wq_sw": wq_sw,
            "wk_sw": _sw3(wk_c.T, HD),
            "wv_sw": _sw3(wv_c.T, HD),
            "wo_sw": wo_sw,
            "cosd": cosd,
            "sind": sind,
            "mask": mask,
        })
    return in_maps


def kernel(x, wq, wk, wv, wo, freqs_cos, freqs_sin, start_pos=0, *,
           _trace=False):
    in_maps = _prep_inputs(np.asarray(x, np.float32), np.asarray(wq, np.float32),
                           np.asarray(wk, np.float32), np.asarray(wv, np.float32),
                           np.asarray(wo, np.float32),
                           np.asarray(freqs_cos, np.float32),
                           np.asarray(freqs_sin, np.float32))
    nc = build_graph()
    res = run_bass_kernel_spmd(nc, in_maps, core_ids=list(range(NCORES)),
                               trace=_trace)
    full = np.concatenate([res.results[c]["out"] for c in range(NCORES)],
                          axis=1)
    out = full.reshape(1, S, D).astype(np.float32)
    if _trace:
        return out, res
    return out
